# revision 34
# baseline (speedup 1.0000x reference)
"""PixelContrastLoss on 8 Trainium2 NeuronCores (Bass/Tile).

Strategy
--------
Host (numpy): the data-dependent hard-anchor sampling (verbatim mirror of the
reference), gather of the sampled features cf [M, D], a stable class-sort of
the M contrast samples, and the final per-row reduction to the scalar loss.

Device (8 cores, SPMD): the O(M^2 D) contrastive matrix work. Rows (anchors)
are sharded round-robin-by-class so every core runs an identical program:
  - logits block = rows x all-columns matmul (fp32r, PE) into PSUM
  - exp(10*l - shift_i) on ACT directly from PSUM, with the row shift
    precomputed on host (shift_i = 10*selfdot_i = the exact row max, since the
    diagonal dominates), accumulating row sums S_all
  - class-sorted columns make the positive mask a contiguous 408-col slice:
    SBUF->SBUF DMA extracts each row-group's slice of E (DMA has no 32-row
    partition-alignment constraint, unlike the compute engines), and the
    raw-logit slice sums come from a tiny matmul against host-precomputed
    per-class feature sums selected by a one-hot (avoids partition-misaligned
    reductions entirely)
  - neg_sum = S_all - S_pos; a deferred ACT Ln(E + ns) pass with per-row bias
    gives the log-denominator slice sums (deferred after all Exp so the ACT
    spline-table set is switched at most twice)
Per-row scalars (4 floats/row) come back to the host, which reproduces the
reference's fp32 semantics exactly - including the degenerate regime where all
off-diagonal exp() underflow (ns == 0), which makes the reference emit NaN via
0 * inf in the masked sum.
"""

import numpy as np
import sys

if "/opt/trn_rl_repo" not in sys.path:
    sys.path.insert(0, "/opt/trn_rl_repo")

TEMPERATURE = 0.1
BASE_TEMPERATURE = 0.07
IGNORE = 255
MAX_SAMPLES = 8192
MAX_VIEWS = 256
HARD_RATIO = 0.5

N_CORES = 8

# fast-path structure constants (canonical problem shape)
FP_T = 160          # anchors
FP_NVIEW = 51       # views per anchor
FP_M = FP_T * FP_NVIEW  # 8160 contrast samples
FP_NCLS = 20        # classes
FP_KC = 408         # columns per class (8 anchors * 51 views)
FP_ROWS_CORE = 1020  # real rows per core
FP_ROWS_PAD = 1024   # padded rows per core (8 blocks of 128)
FP_GROUP = 51        # rows per class per core
FP_NQ = 5            # column quarters per row-block (4 classes each)
FP_QCOLS = 4 * FP_KC  # 1632 cols per quarter
FP_D = 256           # feature dim


def _hard_anchor_sampling_np(y_hat, y, seed=0):
    """Verbatim mirror of the reference's host-side sampling."""
    rng = np.random.RandomState(seed)
    B = y_hat.shape[0]
    classes, total = [], 0
    for ii in range(B):
        cs = [int(c) for c in np.unique(y_hat[ii])
              if c != IGNORE and int((y_hat[ii] == c).sum()) > MAX_VIEWS]
        classes.append(cs)
        total += len(cs)
    if total == 0:
        return None, None, None
    n_view = min(MAX_SAMPLES // total, MAX_VIEWS)
    img_idx, pix_idx, cls_ids = [], [], []
    num_hard_bd = int(HARD_RATIO * n_view)
    num_easy_bd = n_view - num_hard_bd
    for ii in range(B):
        for c in classes[ii]:
            hard = np.nonzero((y_hat[ii] == c) & (y[ii] != c))[0]
            easy = np.nonzero((y_hat[ii] == c) & (y[ii] == c))[0]
            nh, ne = len(hard), len(easy)
            if nh >= num_hard_bd and ne >= num_easy_bd:
                hk = num_hard_bd
                ek = n_view - hk
            elif nh >= num_hard_bd:
                ek = ne
                hk = n_view - ek
            elif ne >= num_easy_bd:
                hk = nh
                ek = n_view - hk
            else:
                raise Exception('this should never be touched! {} {} {}'.format(nh, ne, n_view))
            hs = hard[rng.permutation(nh)[:hk]]
            es = easy[rng.permutation(ne)[:ek]]
            pix_idx.append(np.concatenate([hs, es]))
            img_idx.append(ii)
            cls_ids.append(float(c))
    return (np.asarray(img_idx, np.int32),
            np.stack(pix_idx).astype(np.int32),
            np.asarray(cls_ids, np.float32))


def _block_pieces():
    """Per 128-row block: list of (p0, p1, class) partition sub-ranges.

    Core rows are 20 class-groups of 51 rows (class = row // 51), rows
    1020..1023 are padding attached to class 19.
    """
    out = []
    for b in range(8):
        pieces = []
        r = 128 * b
        end = 128 * (b + 1)
        while r < end:
            c = min(r // FP_GROUP, FP_NCLS - 1)
            if c < FP_NCLS - 1:
                r_end = min((c + 1) * FP_GROUP, end)
            else:
                r_end = end
            pieces.append((r - 128 * b, r_end - 128 * b, c))
            r = r_end
        out.append(pieces)
    return out


_PROGRAM = None


def _build_program():
    """Build the single-core Bass/Tile program (identical across the 8 cores)."""
    import concourse.bass as bass
    import concourse.tile as tile
    from concourse import bacc, mybir
    from contextlib import ExitStack

    f32 = mybir.dt.float32
    f32r = mybir.dt.float32r

    nc = bacc.Bacc("TRN2", target_bir_lowering=False)

    cft_d = nc.dram_tensor("cft", [FP_D, FP_M], f32r, kind="ExternalInput")
    # rows features [*, 0:1024] ++ per-class column-sum features [*, 1024:1044]
    # (the latter feed the sum-of-raw-logits matmul trick)
    rowt_d = nc.dram_tensor("rowt", [FP_D, FP_ROWS_PAD + FP_NCLS], f32r,
                            kind="ExternalInput")
    # [:, 0:8] = -shift per block; [:, 8:168] = one-hot row-class selectors
    nsh_d = nc.dram_tensor("negshift", [128, 8 + 8 * FP_NCLS], f32,
                           kind="ExternalInput")
    out_d = nc.dram_tensor("out", [128, 40], f32, kind="ExternalOutput")

    pieces_by_block = _block_pieces()

    with tile.TileContext(nc) as tc:
        with ExitStack() as ctx:
            cpool = ctx.enter_context(tc.tile_pool(name="cft", bufs=40))
            rpool = ctx.enter_context(tc.tile_pool(name="rowt", bufs=2))
            spool = ctx.enter_context(tc.tile_pool(name="small", bufs=3))
            perst = ctx.enter_context(tc.tile_pool(name="perst", bufs=1))
            epool = ctx.enter_context(tc.tile_pool(name="exp", bufs=6))
            pppool = ctx.enter_context(tc.tile_pool(name="pos", bufs=8))
            dpool = ctx.enter_context(tc.tile_pool(name="dump", bufs=2))
            pspool = ctx.enter_context(
                tc.tile_pool(name="ps", bufs=2, space="PSUM"))

            # ---- persistent tiles + input DMA ----
            # two HWDGE queues exist (SP + ACT): small inputs ride the ACT
            # queue, the big cfT stream rides SP
            rowt = []
            for k in range(2):
                t = rpool.tile([128, FP_ROWS_PAD + FP_NCLS], f32r, tag="rowt")
                nc.scalar.dma_start(t[:, :], rowt_d[128 * k:128 * (k + 1), :])
                rowt.append(t)
            gt = [t[:, FP_ROWS_PAD:FP_ROWS_PAD + FP_NCLS] for t in rowt]

            nshoh = perst.tile([128, 8 + 8 * FP_NCLS], f32, tag="nshoh")
            nc.scalar.dma_start(nshoh[:, :], nsh_d[:, :])
            negshift = nshoh[:, 0:8]
            onehot = nshoh[:, 8:8 + 8 * FP_NCLS]

            outacc = perst.tile([128, 40], f32, tag="outacc")

            # one tile per (k-chunk, class) for fine-grained DMA pipelining
            cft = [[None] * FP_NCLS for _ in range(2)]
            for c in range(FP_NCLS):
                for k in range(2):
                    t = cpool.tile([128, FP_KC], f32r, tag="cft")
                    nc.sync.dma_start(
                        t[:, :],
                        cft_d[128 * k:128 * (k + 1),
                              FP_KC * c:FP_KC * (c + 1)])
                    cft[k][c] = t

            ln_meta = []  # (posE tile, block) for the deferred Ln stage

            # per-block persistent accumulators
            scol_all = perst.tile([128, 8 * FP_NQ], f32, tag="scol_all")
            pose_t = []
            for b in range(8):
                pose = pppool.tile([128, FP_KC], f32, tag="pose")
                pose_t.append(pose)

            # ---- main loop: quarter-major so each streamed cfT chunk feeds
            # ---- 8 row blocks of compute (hides the input DMA) ----
            for qq in range(FP_NQ):
                for b in range(8):
                    pieces = pieces_by_block[b]
                    ps = pspool.tile([128, 4, 512], f32, tag="ps")
                    for k in range(2):
                        lhsT = rowt[k][:, 128 * b:128 * (b + 1)]
                        for ct in range(4):
                            rhs = cft[k][4 * qq + ct][:, :]
                            nc.tensor.matmul(
                                ps[:, ct, 0:FP_KC], lhsT, rhs,
                                start=(k == 0), stop=(k == 1))
                    if qq == 0:
                        # per-class raw-logit sums: rows @ g  -> [128, 20]
                        # parked in the spare columns of bank 3
                        for k in range(2):
                            nc.tensor.matmul(
                                ps[:, 3, 448:448 + FP_NCLS],
                                rowt[k][:, 128 * b:128 * (b + 1)],
                                gt[k],
                                start=(k == 0), stop=(k == 1))

                    # exp(10*l - shift) from PSUM, accumulate row sum
                    e = epool.tile([128, 4, FP_KC], f32, tag="e")
                    nc.scalar.activation(
                        e[:, :, :], ps[:, :, 0:FP_KC],
                        mybir.ActivationFunctionType.Exp,
                        bias=negshift[:, b:b + 1], scale=10.0,
                        accum_out=scol_all[:, FP_NQ * b + qq:FP_NQ * b + qq + 1])
                    # mybir.Instruction of the activation just emitted
                    last_exp_inst = nc.inst_map[next(reversed(nc.inst_map))]

                    if qq == 0:
                        # O1 = sum_c onehot[:, c] * (rows @ g)[:, c]
                        g20 = spool.tile([128, FP_NCLS], f32, tag="g20")
                        nc.vector.tensor_tensor(
                            out=g20[:, :],
                            in0=ps[:, 3, 448:448 + FP_NCLS],
                            in1=onehot[:, FP_NCLS * b:FP_NCLS * (b + 1)],
                            op=mybir.AluOpType.mult)
                        nc.vector.tensor_reduce(
                            out=outacc[:, 5 * b + 0:5 * b + 1],
                            in_=g20[:, :],
                            axis=mybir.AxisListType.X,
                            op=mybir.AluOpType.add)

                    # class-slice E extraction (DMA: no partition alignment
                    # constraint) for pieces living in this quarter
                    for (p0, p1, c) in pieces:
                        if c // 4 != qq:
                            continue
                        ct = c % 4
                        # SWDGE: keep these off the HWDGE input-load queues
                        nc.gpsimd.dma_start(
                            pose_t[b][p0:p1, :], e[p0:p1, ct, :])

            # ---- per-block epilogue: S_all, S_pos, ns ----
            for b in range(8):
                nc.vector.tensor_reduce(
                    out=outacc[:, 5 * b + 3:5 * b + 4],
                    in_=scol_all[:, FP_NQ * b:FP_NQ * (b + 1)],
                    axis=mybir.AxisListType.X,
                    op=mybir.AluOpType.add)
                nc.vector.tensor_reduce(
                    out=outacc[:, 5 * b + 4:5 * b + 5],
                    in_=pose_t[b][:, :],
                    axis=mybir.AxisListType.X,
                    op=mybir.AluOpType.add)
                nc.vector.tensor_sub(
                    outacc[:, 5 * b + 2:5 * b + 3],
                    outacc[:, 5 * b + 3:5 * b + 4],
                    outacc[:, 5 * b + 4:5 * b + 5])
                ln_meta.append((pose_t[b], b))

            # ---- deferred Ln stage (one ACT table set switch at most) ----
            # force every Ln after the final exp so the scheduler cannot
            # interleave them (each Exp<->Ln flip costs a ~2.7us table load)
            from concourse.tile import add_dep_helper
            for pose, b in ln_meta:
                dump = dpool.tile([128, FP_KC], f32, tag="dump")
                nc.scalar.activation(
                    dump[:, :], pose[:, :],
                    mybir.ActivationFunctionType.Ln,
                    bias=outacc[:, 5 * b + 2:5 * b + 3], scale=1.0,
                    accum_out=outacc[:, 5 * b + 1:5 * b + 2])
                ln_inst = nc.inst_map[next(reversed(nc.inst_map))]
                add_dep_helper(ln_inst, last_exp_inst, sync=False,
                               reason="keep Ln after all Exp (ACT table set)")

            nc.sync.dma_start(out_d[:, :], outacc[:, :])

    nc.compile()
    return nc


def _get_program():
    global _PROGRAM
    if _PROGRAM is None:
        _PROGRAM = _build_program()
    return _PROGRAM


def _host_fallback(cf, lab):
    """Pure-numpy fp32 mirror of the reference _contrastive (any structure)."""
    M, D = cf.shape
    f32 = np.float32
    cf = cf.astype(f32)
    temp = f32(TEMPERATURE)
    # row blocks to bound memory
    row_bs = 1024
    mlpp = np.zeros(M, f32)
    pos_m_full = (lab[:, None] == lab[None, :])
    for r0 in range(0, M, row_bs):
        r1 = min(r0 + row_bs, M)
        logits = (cf[r0:r1] @ cf.T).astype(f32) / temp
        logits = logits - logits.max(axis=1, keepdims=True)
        eq = pos_m_full[r0:r1].astype(f32)
        neg = f32(1.0) - eq
        pos = eq.copy()
        pos[np.arange(r0, r1) - r0 + 0, np.arange(r0, r1)] = 0.0
        exp_l = np.exp(logits, dtype=f32)
        neg_sum = (exp_l * neg).sum(axis=1, keepdims=True, dtype=f32)
        log_prob = logits - np.log(exp_l + neg_sum, dtype=f32)
        with np.errstate(invalid="ignore", divide="ignore"):
            mlpp[r0:r1] = (pos * log_prob).sum(axis=1, dtype=f32) / pos.sum(
                axis=1, dtype=f32)
    loss = -(f32(TEMPERATURE) / f32(BASE_TEMPERATURE)) * mlpp
    return np.array(np.mean(loss, dtype=f32), dtype=f32)


def kernel(feats, labels, predict):
    feats = np.asarray(feats, dtype=np.float32)
    labels = np.asarray(labels)
    predict = np.asarray(predict)

    B, C, H, W = feats.shape
    y_np = labels.reshape(B, -1)
    p_np = predict.reshape(B, -1)
    img_idx, pix_idx, cls_ids = _hard_anchor_sampling_np(y_np, p_np, seed=0)
    if img_idx is None:
        return np.array(0.0, dtype=np.float32)

    T, n_view = pix_idx.shape
    feats_flat = feats.transpose(0, 2, 3, 1).reshape(B, H * W, C)
    feats_ = feats_flat[img_idx[:, None], pix_idx]          # [T, n_view, C]
    cf = feats_.transpose(1, 0, 2).reshape(n_view * T, C)   # view-major [M, D]
    M = n_view * T
    lab = np.concatenate([cls_ids] * n_view)                # label per column
    lab_int = lab.astype(np.int64)

    # fast path requires the canonical structure
    uniq, counts = np.unique(lab_int, return_counts=True)
    fast = (T == FP_T and n_view == FP_NVIEW and C == FP_D
            and len(uniq) == FP_NCLS and np.all(counts == FP_KC))
    if not fast:
        return _host_fallback(cf, lab)

    # class-sort columns (stable)
    perm = np.argsort(lab_int, kind="stable")
    cfs = np.ascontiguousarray(cf[perm])                    # [M, D]
    cft_in = np.ascontiguousarray(cfs.T)                    # [D, M]

    sd = np.sum(cfs.astype(np.float64) ** 2, axis=1)
    shift10 = (10.0 * sd).astype(np.float32)                # [M]

    # per-class column-sum features [D, 20] and one-hot selectors [128, 160]
    g = cfs.astype(np.float64).reshape(FP_NCLS, FP_KC, FP_D).sum(axis=1)
    gt_in = np.ascontiguousarray(g.T.astype(np.float32))
    cls_of_row = np.minimum(np.arange(FP_ROWS_PAD) // FP_GROUP, FP_NCLS - 1)
    oh_in = np.zeros((128, 8 * FP_NCLS), np.float32)
    for r in range(FP_ROWS_PAD):
        b, p = divmod(r, 128)
        oh_in[p, FP_NCLS * b + cls_of_row[r]] = 1.0

    # per-core row lists (round-robin by class => identical program structure)
    srows = []
    for k in range(N_CORES):
        idx = np.concatenate([
            np.arange(FP_KC * ci + FP_GROUP * k,
                      FP_KC * ci + FP_GROUP * (k + 1))
            for ci in range(FP_NCLS)])
        srows.append(idx)

    in_maps = []
    shifts_per_core = []
    for k in range(N_CORES):
        rows_k = cfs[srows[k]]                              # [1020, D]
        rowt_k = np.zeros((FP_D, FP_ROWS_PAD + FP_NCLS), np.float32)
        rowt_k[:, :FP_ROWS_CORE] = rows_k.T
        rowt_k[:, FP_ROWS_PAD:] = gt_in
        sh_k = np.zeros(FP_ROWS_PAD, np.float32)
        sh_k[:FP_ROWS_CORE] = shift10[srows[k]]
        nsh_k = np.zeros((128, 8 + 8 * FP_NCLS), np.float32)
        nsh_k[:, 0:8] = (-sh_k).reshape(8, 128).T
        nsh_k[:, 8:] = oh_in
        in_maps.append({
            "cft": cft_in,
            "rowt": rowt_k,
            "negshift": nsh_k,
        })
        shifts_per_core.append(sh_k)

    from concourse.bass_utils import run_bass_kernel_spmd
    nc = _get_program()
    res = run_bass_kernel_spmd(nc, in_maps, list(range(N_CORES)))

    # ---- host finalization (fp32 semantics mirroring the reference) ----
    f32 = np.float32
    ten = f32(10.0)
    kc_f = f32(FP_KC)
    cnt = f32(FP_KC - 1)
    coef = f32(TEMPERATURE / BASE_TEMPERATURE)
    terms = []
    any_ns_zero = False
    for k in range(N_CORES):
        O = res.results[k]["out"].reshape(128, 8, 5)
        # core row r = 128*b + p  ->  O[p, b, :]
        O1 = O[:, :, 0].T.reshape(-1)[:FP_ROWS_CORE]
        O2 = O[:, :, 1].T.reshape(-1)[:FP_ROWS_CORE]
        ns = O[:, :, 2].T.reshape(-1)[:FP_ROWS_CORE]
        sh = shifts_per_core[k][:FP_ROWS_CORE]
        if np.any(np.abs(ns) <= f32(1e-30)):
            any_ns_zero = True
        lp_ii = -np.log(f32(1.0) + ns, dtype=f32)
        # sum over positives of log_prob
        pos_sum = (ten * O1 - kc_f * sh).astype(f32) - O2 - lp_ii
        mlpp = pos_sum / cnt
        terms.append(-coef * mlpp)

    if any_ns_zero:
        # Reference semantics: all off-diagonal exp() underflow -> log(0) =
        # -inf -> 0 * inf = NaN inside the masked row sums -> NaN loss.
        return np.array(np.nan, dtype=np.float32)

    terms = np.concatenate(terms)
    loss = np.mean(terms.astype(np.float64))
    return np.array(loss, dtype=np.float32)


if __name__ == "__main__":
    # smoke test with tiny fake data through the fallback path
    rng = np.random.RandomState(1)
    feats = rng.randn(2, 16, 8, 8).astype(np.float32)
    labels = rng.randint(0, 3, size=(2, 8, 8)).astype(np.int64)
    predict = rng.randint(0, 3, size=(2, 8, 8)).astype(np.int64)
    print(kernel(feats=feats, labels=labels, predict=predict))


# revision 40
# speedup vs baseline: 1.0041x; 1.0041x over previous
"""PixelContrastLoss on 8 Trainium2 NeuronCores (Bass/Tile).

Strategy
--------
Host (numpy): the data-dependent hard-anchor sampling (verbatim mirror of the
reference), gather of the sampled features cf [M, D], a stable class-sort of
the M contrast samples, and the final per-row reduction to the scalar loss.

Device (8 cores, SPMD): the O(M^2 D) contrastive matrix work. Rows (anchors)
are sharded round-robin-by-class so every core runs an identical program:
  - logits block = rows x all-columns matmul (fp32r, PE) into PSUM
  - exp(10*l - shift_i) on ACT directly from PSUM, with the row shift
    precomputed on host (shift_i = 10*selfdot_i = the exact row max, since the
    diagonal dominates), accumulating row sums S_all
  - class-sorted columns make the positive mask a contiguous 408-col slice:
    SBUF->SBUF DMA extracts each row-group's slice of E (DMA has no 32-row
    partition-alignment constraint, unlike the compute engines), and the
    raw-logit slice sums come from a tiny matmul against host-precomputed
    per-class feature sums selected by a one-hot (avoids partition-misaligned
    reductions entirely)
  - neg_sum = S_all - S_pos; a deferred ACT Ln(E + ns) pass with per-row bias
    gives the log-denominator slice sums (deferred after all Exp so the ACT
    spline-table set is switched at most twice)
Per-row scalars (4 floats/row) come back to the host, which reproduces the
reference's fp32 semantics exactly - including the degenerate regime where all
off-diagonal exp() underflow (ns == 0), which makes the reference emit NaN via
0 * inf in the masked sum.
"""

import numpy as np
import sys

if "/opt/trn_rl_repo" not in sys.path:
    sys.path.insert(0, "/opt/trn_rl_repo")

TEMPERATURE = 0.1
BASE_TEMPERATURE = 0.07
IGNORE = 255
MAX_SAMPLES = 8192
MAX_VIEWS = 256
HARD_RATIO = 0.5

N_CORES = 8

# fast-path structure constants (canonical problem shape)
FP_T = 160          # anchors
FP_NVIEW = 51       # views per anchor
FP_M = FP_T * FP_NVIEW  # 8160 contrast samples
FP_NCLS = 20        # classes
FP_KC = 408         # columns per class (8 anchors * 51 views)
FP_ROWS_CORE = 1020  # real rows per core
FP_ROWS_PAD = 1024   # padded rows per core (8 blocks of 128)
FP_GROUP = 51        # rows per class per core
FP_NQ = 5            # column quarters per row-block (4 classes each)
FP_QCOLS = 4 * FP_KC  # 1632 cols per quarter
FP_D = 256           # feature dim


def _hard_anchor_sampling_np(y_hat, y, seed=0):
    """Verbatim mirror of the reference's host-side sampling."""
    rng = np.random.RandomState(seed)
    B = y_hat.shape[0]
    classes, total = [], 0
    for ii in range(B):
        cs = [int(c) for c in np.unique(y_hat[ii])
              if c != IGNORE and int((y_hat[ii] == c).sum()) > MAX_VIEWS]
        classes.append(cs)
        total += len(cs)
    if total == 0:
        return None, None, None
    n_view = min(MAX_SAMPLES // total, MAX_VIEWS)
    img_idx, pix_idx, cls_ids = [], [], []
    num_hard_bd = int(HARD_RATIO * n_view)
    num_easy_bd = n_view - num_hard_bd
    for ii in range(B):
        for c in classes[ii]:
            hard = np.nonzero((y_hat[ii] == c) & (y[ii] != c))[0]
            easy = np.nonzero((y_hat[ii] == c) & (y[ii] == c))[0]
            nh, ne = len(hard), len(easy)
            if nh >= num_hard_bd and ne >= num_easy_bd:
                hk = num_hard_bd
                ek = n_view - hk
            elif nh >= num_hard_bd:
                ek = ne
                hk = n_view - ek
            elif ne >= num_easy_bd:
                hk = nh
                ek = n_view - hk
            else:
                raise Exception('this should never be touched! {} {} {}'.format(nh, ne, n_view))
            hs = hard[rng.permutation(nh)[:hk]]
            es = easy[rng.permutation(ne)[:ek]]
            pix_idx.append(np.concatenate([hs, es]))
            img_idx.append(ii)
            cls_ids.append(float(c))
    return (np.asarray(img_idx, np.int32),
            np.stack(pix_idx).astype(np.int32),
            np.asarray(cls_ids, np.float32))


def _block_pieces():
    """Per 128-row block: list of (p0, p1, class) partition sub-ranges.

    Core rows are 20 class-groups of 51 rows (class = row // 51), rows
    1020..1023 are padding attached to class 19.
    """
    out = []
    for b in range(8):
        pieces = []
        r = 128 * b
        end = 128 * (b + 1)
        while r < end:
            c = min(r // FP_GROUP, FP_NCLS - 1)
            if c < FP_NCLS - 1:
                r_end = min((c + 1) * FP_GROUP, end)
            else:
                r_end = end
            pieces.append((r - 128 * b, r_end - 128 * b, c))
            r = r_end
        out.append(pieces)
    return out


_PROGRAM = None


def _build_program():
    """Build the single-core Bass/Tile program (identical across the 8 cores)."""
    import concourse.bass as bass
    import concourse.tile as tile
    from concourse import bacc, mybir
    from contextlib import ExitStack

    f32 = mybir.dt.float32
    f32r = mybir.dt.float32r

    class _BaccOneActSet(bacc.Bacc):
        """Force Exp and Ln onto the combined natural_log_exp_and_others
        ACT table set (one table load instead of two; makes Exp<->Ln
        interleave free). Same logic as Bacc.insert_act_table_loads but
        with the exp-only / ln-only sets blanked (list indices preserved
        because act_func_set_id is positional)."""

        def insert_act_table_loads(self):
            import bass_rust as _bass_rust
            from concourse.hw_specs import get_activation_tables
            has_activation = any(
                isinstance(i, mybir.InstActivation)
                for b in self.main_func.blocks
                for i in b.instructions
            )
            if not has_activation:
                return
            tables = []
            for name, s in get_activation_tables(self.m.arch).items():
                if name in ("exp_and_others", "natural_log",
                            "exp_and_friends"):
                    s = set()
                tables.append((name, s))
            _bass_rust.insert_act_table_loads(self, tables)

    nc = _BaccOneActSet("TRN2", target_bir_lowering=False)

    cft_d = nc.dram_tensor("cft", [FP_D, FP_M], f32r, kind="ExternalInput")
    # rows features [*, 0:1024] ++ per-class column-sum features [*, 1024:1044]
    # (the latter feed the sum-of-raw-logits matmul trick)
    rowt_d = nc.dram_tensor("rowt", [FP_D, FP_ROWS_PAD + FP_NCLS], f32r,
                            kind="ExternalInput")
    # [:, 0:8] = -shift per block; [:, 8:168] = one-hot row-class selectors
    nsh_d = nc.dram_tensor("negshift", [128, 8 + 8 * FP_NCLS], f32,
                           kind="ExternalInput")
    out_d = nc.dram_tensor("out", [128, 40], f32, kind="ExternalOutput")

    pieces_by_block = _block_pieces()

    with tile.TileContext(nc) as tc:
        with ExitStack() as ctx:
            cpool = ctx.enter_context(tc.tile_pool(name="cft", bufs=40))
            rpool = ctx.enter_context(tc.tile_pool(name="rowt", bufs=2))
            spool = ctx.enter_context(tc.tile_pool(name="small", bufs=3))
            perst = ctx.enter_context(tc.tile_pool(name="perst", bufs=1))
            epool = ctx.enter_context(tc.tile_pool(name="exp", bufs=8))
            pppool = ctx.enter_context(tc.tile_pool(name="pos", bufs=8))
            dpool = ctx.enter_context(tc.tile_pool(name="dump", bufs=2))
            pspool = ctx.enter_context(
                tc.tile_pool(name="ps", bufs=2, space="PSUM"))

            # ---- persistent tiles + input DMA ----
            # two HWDGE queues exist (SP + ACT): small inputs ride the ACT
            # queue, the big cfT stream rides SP
            rowt = []
            for k in range(2):
                t = rpool.tile([128, FP_ROWS_PAD + FP_NCLS], f32r, tag="rowt")
                nc.scalar.dma_start(t[:, :], rowt_d[128 * k:128 * (k + 1), :])
                rowt.append(t)
            gt = [t[:, FP_ROWS_PAD:FP_ROWS_PAD + FP_NCLS] for t in rowt]

            nshoh = perst.tile([128, 8 + 8 * FP_NCLS], f32, tag="nshoh")
            nc.scalar.dma_start(nshoh[:, :], nsh_d[:, :])
            negshift = nshoh[:, 0:8]
            onehot = nshoh[:, 8:8 + 8 * FP_NCLS]

            outacc = perst.tile([128, 40], f32, tag="outacc")

            # one tile per (k-chunk, class) for fine-grained DMA pipelining
            cft = [[None] * FP_NCLS for _ in range(2)]
            for c in range(FP_NCLS):
                for k in range(2):
                    t = cpool.tile([128, FP_KC], f32r, tag="cft")
                    nc.sync.dma_start(
                        t[:, :],
                        cft_d[128 * k:128 * (k + 1),
                              FP_KC * c:FP_KC * (c + 1)])
                    cft[k][c] = t

            ln_meta = []  # (posE tile, block) for the deferred Ln stage

            # per-block persistent accumulators
            scol_all = perst.tile([128, 8 * FP_NQ], f32, tag="scol_all")
            pose_t = []
            for b in range(8):
                pose = pppool.tile([128, FP_KC], f32, tag="pose")
                pose_t.append(pose)

            # ---- main loop: quarter-major so each streamed cfT chunk feeds
            # ---- 8 row blocks of compute (hides the input DMA) ----
            for qq in range(FP_NQ):
                for b in range(8):
                    pieces = pieces_by_block[b]
                    ps = pspool.tile([128, 4, 512], f32, tag="ps")
                    for k in range(2):
                        lhsT = rowt[k][:, 128 * b:128 * (b + 1)]
                        for ct in range(4):
                            rhs = cft[k][4 * qq + ct][:, :]
                            nc.tensor.matmul(
                                ps[:, ct, 0:FP_KC], lhsT, rhs,
                                start=(k == 0), stop=(k == 1))
                    if qq == 0:
                        # per-class raw-logit sums: rows @ g  -> [128, 20]
                        # parked in the spare columns of bank 3
                        for k in range(2):
                            nc.tensor.matmul(
                                ps[:, 3, 448:448 + FP_NCLS],
                                rowt[k][:, 128 * b:128 * (b + 1)],
                                gt[k],
                                start=(k == 0), stop=(k == 1))

                    # exp(10*l - shift) from PSUM, accumulate row sum
                    e = epool.tile([128, 4, FP_KC], f32, tag="e")
                    nc.scalar.activation(
                        e[:, :, :], ps[:, :, 0:FP_KC],
                        mybir.ActivationFunctionType.Exp,
                        bias=negshift[:, b:b + 1], scale=10.0,
                        accum_out=scol_all[:, FP_NQ * b + qq:FP_NQ * b + qq + 1])
                    # mybir.Instruction of the activation just emitted
                    last_exp_inst = nc.inst_map[next(reversed(nc.inst_map))]

                    if qq == 0:
                        # O1 = sum_c onehot[:, c] * (rows @ g)[:, c]
                        g20 = spool.tile([128, FP_NCLS], f32, tag="g20")
                        nc.vector.tensor_tensor(
                            out=g20[:, :],
                            in0=ps[:, 3, 448:448 + FP_NCLS],
                            in1=onehot[:, FP_NCLS * b:FP_NCLS * (b + 1)],
                            op=mybir.AluOpType.mult)
                        nc.vector.tensor_reduce(
                            out=outacc[:, 5 * b + 0:5 * b + 1],
                            in_=g20[:, :],
                            axis=mybir.AxisListType.X,
                            op=mybir.AluOpType.add)

                    # class-slice E extraction (DMA: no partition alignment
                    # constraint) for pieces living in this quarter
                    for (p0, p1, c) in pieces:
                        if c // 4 != qq:
                            continue
                        ct = c % 4
                        # SWDGE: keep these off the HWDGE input-load queues
                        nc.gpsimd.dma_start(
                            pose_t[b][p0:p1, :], e[p0:p1, ct, :])

            # ---- per-block epilogue: S_all, S_pos, ns ----
            for b in range(8):
                nc.vector.tensor_reduce(
                    out=outacc[:, 5 * b + 3:5 * b + 4],
                    in_=scol_all[:, FP_NQ * b:FP_NQ * (b + 1)],
                    axis=mybir.AxisListType.X,
                    op=mybir.AluOpType.add)
                nc.vector.tensor_reduce(
                    out=outacc[:, 5 * b + 4:5 * b + 5],
                    in_=pose_t[b][:, :],
                    axis=mybir.AxisListType.X,
                    op=mybir.AluOpType.add)
                nc.vector.tensor_sub(
                    outacc[:, 5 * b + 2:5 * b + 3],
                    outacc[:, 5 * b + 3:5 * b + 4],
                    outacc[:, 5 * b + 4:5 * b + 5])
                ln_meta.append((pose_t[b], b))

            # ---- deferred Ln stage (one ACT table set switch at most) ----
            # force every Ln after the final exp so the scheduler cannot
            # interleave them (each Exp<->Ln flip costs a ~2.7us table load)
            from concourse.tile import add_dep_helper
            for pose, b in ln_meta:
                dump = dpool.tile([128, FP_KC], f32, tag="dump")
                nc.scalar.activation(
                    dump[:, :], pose[:, :],
                    mybir.ActivationFunctionType.Ln,
                    bias=outacc[:, 5 * b + 2:5 * b + 3], scale=1.0,
                    accum_out=outacc[:, 5 * b + 1:5 * b + 2])
                ln_inst = nc.inst_map[next(reversed(nc.inst_map))]
                add_dep_helper(ln_inst, last_exp_inst, sync=False,
                               reason="keep Ln after all Exp (ACT table set)")

            nc.sync.dma_start(out_d[:, :], outacc[:, :])

    nc.compile()
    return nc


def _get_program():
    global _PROGRAM
    if _PROGRAM is None:
        _PROGRAM = _build_program()
    return _PROGRAM


def _host_fallback(cf, lab):
    """Pure-numpy fp32 mirror of the reference _contrastive (any structure)."""
    M, D = cf.shape
    f32 = np.float32
    cf = cf.astype(f32)
    temp = f32(TEMPERATURE)
    # row blocks to bound memory
    row_bs = 1024
    mlpp = np.zeros(M, f32)
    pos_m_full = (lab[:, None] == lab[None, :])
    for r0 in range(0, M, row_bs):
        r1 = min(r0 + row_bs, M)
        logits = (cf[r0:r1] @ cf.T).astype(f32) / temp
        logits = logits - logits.max(axis=1, keepdims=True)
        eq = pos_m_full[r0:r1].astype(f32)
        neg = f32(1.0) - eq
        pos = eq.copy()
        pos[np.arange(r0, r1) - r0 + 0, np.arange(r0, r1)] = 0.0
        exp_l = np.exp(logits, dtype=f32)
        neg_sum = (exp_l * neg).sum(axis=1, keepdims=True, dtype=f32)
        log_prob = logits - np.log(exp_l + neg_sum, dtype=f32)
        with np.errstate(invalid="ignore", divide="ignore"):
            mlpp[r0:r1] = (pos * log_prob).sum(axis=1, dtype=f32) / pos.sum(
                axis=1, dtype=f32)
    loss = -(f32(TEMPERATURE) / f32(BASE_TEMPERATURE)) * mlpp
    return np.array(np.mean(loss, dtype=f32), dtype=f32)


def kernel(feats, labels, predict):
    feats = np.asarray(feats, dtype=np.float32)
    labels = np.asarray(labels)
    predict = np.asarray(predict)

    B, C, H, W = feats.shape
    y_np = labels.reshape(B, -1)
    p_np = predict.reshape(B, -1)
    img_idx, pix_idx, cls_ids = _hard_anchor_sampling_np(y_np, p_np, seed=0)
    if img_idx is None:
        return np.array(0.0, dtype=np.float32)

    T, n_view = pix_idx.shape
    feats_flat = feats.transpose(0, 2, 3, 1).reshape(B, H * W, C)
    feats_ = feats_flat[img_idx[:, None], pix_idx]          # [T, n_view, C]
    cf = feats_.transpose(1, 0, 2).reshape(n_view * T, C)   # view-major [M, D]
    M = n_view * T
    lab = np.concatenate([cls_ids] * n_view)                # label per column
    lab_int = lab.astype(np.int64)

    # fast path requires the canonical structure
    uniq, counts = np.unique(lab_int, return_counts=True)
    fast = (T == FP_T and n_view == FP_NVIEW and C == FP_D
            and len(uniq) == FP_NCLS and np.all(counts == FP_KC))
    if not fast:
        return _host_fallback(cf, lab)

    # class-sort columns (stable)
    perm = np.argsort(lab_int, kind="stable")
    cfs = np.ascontiguousarray(cf[perm])                    # [M, D]
    cft_in = np.ascontiguousarray(cfs.T)                    # [D, M]

    sd = np.sum(cfs.astype(np.float64) ** 2, axis=1)
    shift10 = (10.0 * sd).astype(np.float32)                # [M]

    # per-class column-sum features [D, 20] and one-hot selectors [128, 160]
    g = cfs.astype(np.float64).reshape(FP_NCLS, FP_KC, FP_D).sum(axis=1)
    gt_in = np.ascontiguousarray(g.T.astype(np.float32))
    cls_of_row = np.minimum(np.arange(FP_ROWS_PAD) // FP_GROUP, FP_NCLS - 1)
    oh_in = np.zeros((128, 8 * FP_NCLS), np.float32)
    for r in range(FP_ROWS_PAD):
        b, p = divmod(r, 128)
        oh_in[p, FP_NCLS * b + cls_of_row[r]] = 1.0

    # per-core row lists (round-robin by class => identical program structure)
    srows = []
    for k in range(N_CORES):
        idx = np.concatenate([
            np.arange(FP_KC * ci + FP_GROUP * k,
                      FP_KC * ci + FP_GROUP * (k + 1))
            for ci in range(FP_NCLS)])
        srows.append(idx)

    in_maps = []
    shifts_per_core = []
    for k in range(N_CORES):
        rows_k = cfs[srows[k]]                              # [1020, D]
        rowt_k = np.zeros((FP_D, FP_ROWS_PAD + FP_NCLS), np.float32)
        rowt_k[:, :FP_ROWS_CORE] = rows_k.T
        rowt_k[:, FP_ROWS_PAD:] = gt_in
        sh_k = np.zeros(FP_ROWS_PAD, np.float32)
        sh_k[:FP_ROWS_CORE] = shift10[srows[k]]
        nsh_k = np.zeros((128, 8 + 8 * FP_NCLS), np.float32)
        nsh_k[:, 0:8] = (-sh_k).reshape(8, 128).T
        nsh_k[:, 8:] = oh_in
        in_maps.append({
            "cft": cft_in,
            "rowt": rowt_k,
            "negshift": nsh_k,
        })
        shifts_per_core.append(sh_k)

    from concourse.bass_utils import run_bass_kernel_spmd
    nc = _get_program()
    res = run_bass_kernel_spmd(nc, in_maps, list(range(N_CORES)))

    # ---- host finalization (fp32 semantics mirroring the reference) ----
    f32 = np.float32
    ten = f32(10.0)
    kc_f = f32(FP_KC)
    cnt = f32(FP_KC - 1)
    coef = f32(TEMPERATURE / BASE_TEMPERATURE)
    terms = []
    any_ns_zero = False
    for k in range(N_CORES):
        O = res.results[k]["out"].reshape(128, 8, 5)
        # core row r = 128*b + p  ->  O[p, b, :]
        O1 = O[:, :, 0].T.reshape(-1)[:FP_ROWS_CORE]
        O2 = O[:, :, 1].T.reshape(-1)[:FP_ROWS_CORE]
        ns = O[:, :, 2].T.reshape(-1)[:FP_ROWS_CORE]
        sh = shifts_per_core[k][:FP_ROWS_CORE]
        if np.any(np.abs(ns) <= f32(1e-30)):
            any_ns_zero = True
        lp_ii = -np.log(f32(1.0) + ns, dtype=f32)
        # sum over positives of log_prob
        pos_sum = (ten * O1 - kc_f * sh).astype(f32) - O2 - lp_ii
        mlpp = pos_sum / cnt
        terms.append(-coef * mlpp)

    if any_ns_zero:
        # Reference semantics: all off-diagonal exp() underflow -> log(0) =
        # -inf -> 0 * inf = NaN inside the masked row sums -> NaN loss.
        return np.array(np.nan, dtype=np.float32)

    terms = np.concatenate(terms)
    loss = np.mean(terms.astype(np.float64))
    return np.array(loss, dtype=np.float32)


if __name__ == "__main__":
    # smoke test with tiny fake data through the fallback path
    rng = np.random.RandomState(1)
    feats = rng.randn(2, 16, 8, 8).astype(np.float32)
    labels = rng.randint(0, 3, size=(2, 8, 8)).astype(np.int64)
    predict = rng.randint(0, 3, size=(2, 8, 8)).astype(np.int64)
    print(kernel(feats=feats, labels=labels, predict=predict))
